# revision 16
# baseline (speedup 1.0000x reference)
"""DSV4 Main-KV projection kernel for 8 Trainium2 NeuronCores.

Computation (see reference): kv = x @ wkv.T ; RMSNorm(D=512) * rms_weight;
RoPE on last 64 dims; per-64-block fp8 quant-dequant simulation on first
448 dims. Data-parallel over the 16384 tokens (2048 per core).

x and wkv are cast to bf16 on the host (matmul rate on TRN2 is identical
to f32r at free-dim 512, but HBM traffic halves); all post-GEMM math stays
fp32. Inputs are pre-packed so every DMA is contiguous per partition.

Self-contained: hardcodes shapes; only imports the system toolchain.
"""
import sys
sys.path.insert(0, '/opt/trn_rl_repo')

import numpy as np
import ml_dtypes
from contextlib import ExitStack

import concourse.bass as bass
import concourse.mybir as mybir
import concourse.tile as tile
from concourse.bass_utils import run_bass_kernel_spmd
import bass_rust

dt = mybir.dt
BF16 = ml_dtypes.bfloat16

B, S, H, D = 4, 4096, 4096, 512
RD = 64                 # rope dims (last)
QD = D - RD             # quantized dims (first 448)
NBLK = QD // 64         # 7 quant blocks
BLK = 64
FP8_MAX = 448.0
EPS = 1e-6
ROPE_BASE = 10000.0
NCORES = 8
TOK = (B * S) // NCORES          # 2048 tokens per core
TT = 128                         # tokens per tile
NT = TOK // TT                   # 16 tiles per core
KC = H // 128                    # 32 contraction chunks
M_RND = 12582912.0               # 1.5 * 2**23: fp32 round-to-int magic

_compiled = {}


# ---------------------------------------------------------------------------
# walrus in this container caps sync waits at 1/instruction (2 for
# EventSemaphore); Tile emits more. Split the excess into preceding
# single-wait NoOps on the same engine.
def _split_multi_waits(nc):
    ctr = 0
    for f in nc.m.functions:
        for b in f.blocks:
            out, changed = [], False
            for inst in b.instructions:
                si = inst.sync_info
                cap = 2 if type(inst).__name__ == 'InstEventSemaphore' else 1
                if si is not None and len(si.on_wait) > cap:
                    waits = list(si.on_wait)
                    for w in waits[:-cap]:
                        ctr += 1
                        nop = mybir.InstNoOp(name=f'wsplit-{ctr}', ins=[], outs=[])
                        nop.engine = inst.engine
                        nop.sync_info = bass_rust.SyncInfo(on_wait=[w], on_update=[])
                        out.append(nop)
                    inst.sync_info = bass_rust.SyncInfo(on_wait=waits[-cap:],
                                                        on_update=si.on_update)
                    changed = True
                out.append(inst)
            if changed:
                b.instructions = out
    return ctr


def _build_nc(reps=1):
    nc = bass.Bass('TRN2', target_bir_lowering=False, debug=False)
    Alu = mybir.AluOpType
    Act = mybir.ActivationFunctionType

    # pre-packed inputs: contiguous per partition (see _host_prep)
    xb = nc.dram_tensor('xb', [128, NT * KC * TT], dt.bfloat16,
                        kind='ExternalInput').ap()
    wb = nc.dram_tensor('wb', [128, KC * D], dt.bfloat16,
                        kind='ExternalInput').ap()
    rmsr = nc.dram_tensor('rmsr', [128, D], dt.float32, kind='ExternalInput').ap()
    c2d = nc.dram_tensor('c2d', [128, NT * RD], dt.float32,
                         kind='ExternalInput').ap()
    s2d = nc.dram_tensor('s2d', [128, NT * RD], dt.float32,
                         kind='ExternalInput').ap()
    out = nc.dram_tensor('out', [TOK, D], dt.bfloat16, kind='ExternalOutput').ap()

    with tile.TileContext(nc) as tc, ExitStack() as ctx:
        const = ctx.enter_context(tc.tile_pool(name='const', bufs=1))
        xpool = ctx.enter_context(tc.tile_pool(name='xp', bufs=3))
        kpool = ctx.enter_context(tc.tile_pool(name='kp', bufs=3))
        opool = ctx.enter_context(tc.tile_pool(name='op', bufs=3))
        spool = ctx.enter_context(tc.tile_pool(name='sp', bufs=2))
        psum = ctx.enter_context(tc.tile_pool(name='ps', bufs=8, space='PSUM'))

        def xt_dma(t, name=None):
            # x tile [128, KC, TT] bf16 — one contiguous 8KB chunk/partition
            xt = xpool.tile([128, KC, TT], dt.bfloat16,
                            name=name or f'xt_{t}', tag='xt')
            nc.sync.dma_start(
                xt[:], xb[:, t * KC * TT:(t + 1) * KC * TT]
                .rearrange('p (c m) -> p c m', c=KC))
            return xt

        # DMA issue is split across the two HW-DGE engines: descriptor
        # generation costs ~0.6us per dma_start, serial per engine. Weights +
        # tables go through the Activation engine (idle at start); x tiles and
        # outputs through Sync. Weight chunks of 4 k-slices pace tile 0's
        # matmuls while the weight stream is still arriving.
        WCH = 4                           # k-slices per weight DMA chunk
        # tile-0 x arrives as 8 independent sub-tiles so the first matmuls
        # can start as soon as the first 128KB (+ first weight chunk) lands
        xt0c = []
        for c in range(KC // WCH):
            xc = xpool.tile([128, WCH, TT], dt.bfloat16, name=f'xt0c{c}')
            nc.sync.dma_start(
                xc[:], xb[:, c * WCH * TT:(c + 1) * WCH * TT]
                .rearrange('p (c m) -> p c m', c=WCH))
            xt0c.append(xc)
        wch = []
        for c in range(KC // WCH):
            wc = const.tile([128, WCH, D], dt.bfloat16, name=f'wc{c}')
            nc.scalar.dma_start(
                wc[:], wb[:, c * WCH * D:(c + 1) * WCH * D]
                .rearrange('p (j d) -> p j d', j=WCH))
            wch.append(wc)
        wts = [wch[k // WCH][:, k % WCH, :] for k in range(KC)]
        # replicated rms weight [128, D]
        rms = const.tile([128, D], dt.float32, name='rms')
        nc.scalar.dma_start(rms[:], rmsr)
        # rope tables [128, NT, RD]
        c2 = const.tile([128, NT, RD], dt.float32, name='c2')
        nc.scalar.dma_start(c2[:], c2d.rearrange('p (t f) -> p t f', t=NT))
        s2 = const.tile([128, NT, RD], dt.float32, name='s2')
        nc.scalar.dma_start(s2[:], s2d.rearrange('p (t f) -> p t f', t=NT))
        xt1 = xt_dma(1)

        for rep in range(reps):
         for t in range(NT):
             if rep == 0 and t == 0:
                 xap = lambda k: xt0c[k // WCH][:, k % WCH, :]
             elif rep == 0 and t == 1:
                 xap = lambda k, _x=xt1: _x[:, k, :]
             else:
                 xt = xt_dma(t, name=f'xt_{rep}_{t}')
                 xap = lambda k, _x=xt: _x[:, k, :]

             ps = psum.tile([TT, D], dt.float32, name=f'ps{rep}_{t}', tag='ps')
             for k in range(KC):
                 nc.tensor.matmul(ps[:], xap(k), wts[k],
                                  start=(k == 0), stop=(k == KC - 1))

             # --- RMSNorm ---  var = mean(kv^2) via scale-folded Square;
             # EPS (1e-6) is negligible against var ~ 1 and is dropped.
             sq = spool.tile([TT, D], dt.float32, name=f'sq{rep}_{t}', tag='sq')
             var = spool.tile([TT, 1], dt.float32, name=f'var{rep}_{t}', tag='var')
             nc.scalar.activation(sq[:], ps[:], Act.Square,
                                  scale=float(1.0 / np.sqrt(D)), accum_out=var[:])
             rv = spool.tile([TT, 1], dt.float32, name=f'rv{rep}_{t}', tag='rv')
             nc.vector.reciprocal(rv[:], var[:])
             rstd = spool.tile([TT, 1], dt.float32, name=f'rstd{rep}_{t}', tag='rstd')
             nc.scalar.activation(rstd[:], rv[:], Act.Sqrt)
             # kvw = (kv * rstd) * rms
             kvw = kpool.tile([TT, D], dt.float32, name=f'kvw{rep}_{t}', tag='kvw')
             nc.vector.scalar_tensor_tensor(kvw[:], ps[:], rstd[:], rms[:],
                                            op0=Alu.mult, op1=Alu.mult)

             ot = opool.tile([TT, D], dt.bfloat16, name=f'ot{rep}_{t}', tag='ot')

             # --- quant-dequant on [:, :448] ---
             amax = spool.tile([TT, NBLK], dt.float32, name=f'amax{rep}_{t}', tag='amax')
             nc.vector.tensor_reduce(
                 amax[:], kvw[:, 0:QD].rearrange('p (b k) -> p b k', k=BLK),
                 axis=mybir.AxisListType.X, op=Alu.max, apply_absolute_value=True)
             sc = spool.tile([TT, NBLK], dt.float32, name=f'sc{rep}_{t}', tag='sc')
             nc.vector.tensor_scalar(sc[:], amax[:], 1e-4, FP8_MAX / 127.0 / FP8_MAX,
                                     op0=Alu.max, op1=Alu.mult)  # s = amax'/127
             rq = spool.tile([TT, NBLK], dt.float32, name=f'rq{rep}_{t}', tag='rq')
             nc.vector.reciprocal(rq[:], sc[:])                  # 127/amax'
             vq = kpool.tile([TT, QD], dt.float32, name=f'vq{rep}_{t}', tag='vq')
             rq_b = bass.AP(tensor=rq.tensor, offset=rq[:].offset,
                            ap=[[rq[:].ap[0][0], TT], [1, NBLK], [0, BLK]])
             nc.vector.tensor_tensor(
                 vq[:].rearrange('p (b k) -> p b k', k=BLK),
                 kvw[:, 0:QD].rearrange('p (b k) -> p b k', k=BLK),
                 rq_b, op=Alu.mult)
             nc.vector.tensor_scalar(vq[:], vq[:], M_RND, M_RND,
                                     op0=Alu.add, op1=Alu.subtract)
             sc_b = bass.AP(tensor=sc.tensor, offset=sc[:].offset,
                            ap=[[sc[:].ap[0][0], TT], [1, NBLK], [0, BLK]])
             nc.vector.tensor_tensor(
                 ot[:, 0:QD].rearrange('p (b k) -> p b k', k=BLK),
                 vq[:].rearrange('p (b k) -> p b k', k=BLK),
                 sc_b, op=Alu.mult)

             # --- rope on [:, 448:] (on GpSimd, concurrent with quant on DVE) ---
             # out = kvw_rope * c2 + pairswap(kvw_rope) * s2
             sw = spool.tile([TT, RD], dt.float32, name=f'sw{rep}_{t}', tag='sw')
             src_swap = bass.AP(tensor=kvw.tensor, offset=kvw[:].offset + QD + 1,
                                ap=[[kvw[:].ap[0][0], TT], [2, RD // 2], [-1, 2]])
             nc.gpsimd.tensor_copy(sw[:].rearrange('p (a b) -> p a b', b=2), src_swap)
             t1 = spool.tile([TT, RD], dt.float32, name=f't1{rep}_{t}', tag='t1')
             nc.gpsimd.tensor_tensor(t1[:], kvw[:, QD:D], c2[:, t, :], op=Alu.mult)
             t2 = spool.tile([TT, RD], dt.float32, name=f't2{rep}_{t}', tag='t2')
             nc.gpsimd.tensor_tensor(t2[:], sw[:], s2[:, t, :], op=Alu.mult)
             nc.gpsimd.tensor_tensor(ot[:, QD:D], t1[:], t2[:], op=Alu.add)

             nc.sync.dma_start(out[t * TT:(t + 1) * TT, :], ot[:])

    _split_multi_waits(nc)
    return nc


def _host_prep(x, wkv_weight, rms_weight):
    """Shard + pack on host; build rope tables. Returns per-core in_maps.

    Packed layouts (all contiguous per partition):
      xb [128, NT*KC*TT] bf16 : xb[p, (t*KC+c)*TT+m] = x[tok0 + t*TT+m, c*128+p]
      wb [128, KC*D]     bf16 : wb[p, c*D+d]         = wkv[d, c*128+p]
      c2d/s2d [128, NT*RD] f32: c2d[p, t*RD+f]       = table[pos(t*TT+p), f]
    """
    xf = np.ascontiguousarray(x, dtype=np.float32).reshape(B * S, H)
    wb = np.ascontiguousarray(
        wkv_weight.astype(np.float32).T.reshape(KC, 128, D).transpose(1, 0, 2)
        .astype(BF16).reshape(128, KC * D))
    rmsr = np.broadcast_to(np.asarray(rms_weight, np.float32)[None, :],
                           (128, D)).copy()

    # rope tables for all positions: duplicated cos / sign-folded sin
    freqs = 1.0 / ROPE_BASE ** (np.arange(0, RD, 2, dtype=np.float64) / RD)
    tpos = np.arange(S, dtype=np.float64)
    ang = np.outer(tpos, freqs)                                        # [S, 32]
    cos = np.cos(ang).astype(np.float32)
    sin = np.sin(ang).astype(np.float32)
    c2 = np.empty((S, RD), np.float32)
    s2 = np.empty((S, RD), np.float32)
    c2[:, 0::2] = cos
    c2[:, 1::2] = cos
    s2[:, 0::2] = -sin          # even out: a*cos - b*sin ; sw[even]=b
    s2[:, 1::2] = sin           # odd  out: a*sin + b*cos ; sw[odd]=a

    in_maps = []
    for c in range(NCORES):
        tok0 = c * TOK
        # [TOK, H] -> [t, m, kc, p] -> [p, t, kc, m] bf16, contiguous
        xs = (xf[tok0:tok0 + TOK, :].reshape(NT, TT, KC, 128)
              .transpose(3, 0, 2, 1).astype(BF16).reshape(128, NT * KC * TT))
        spos = (np.arange(tok0, tok0 + TOK)) % S
        c2c = np.ascontiguousarray(
            c2[spos].reshape(NT, TT, RD).transpose(1, 0, 2)
            .reshape(128, NT * RD))
        s2c = np.ascontiguousarray(
            s2[spos].reshape(NT, TT, RD).transpose(1, 0, 2)
            .reshape(128, NT * RD))
        in_maps.append({
            'xb': xs,
            'wb': wb,
            'rmsr': rmsr,
            'c2d': c2c,
            's2d': s2c,
        })
    return in_maps


def kernel(x, wkv_weight, rms_weight, _trace=False, _trace_kwargs=None):
    in_maps = _host_prep(x, wkv_weight, rms_weight)
    if 'nc' not in _compiled:
        _compiled['nc'] = _build_nc()
    nc = _compiled['nc']
    kw = {}
    if _trace:
        kw = dict(trace=True, trace_cores=[0], **(_trace_kwargs or {}))
    res = run_bass_kernel_spmd(nc, in_maps, core_ids=list(range(NCORES)), **kw)
    outs = [r['out'] for r in res.results]
    full = np.concatenate(outs, axis=0).reshape(B, S, D).astype(np.float32)
    kernel._last_results = res
    return full


if __name__ == '__main__':
    rng = np.random.default_rng(0)
    x = rng.standard_normal((B, S, H), dtype=np.float32)
    w = (rng.standard_normal((D, H), dtype=np.float32) * H ** -0.5).astype(np.float32)
    rw = np.ones((D,), np.float32)
    o = kernel(x, w, rw)
    print('out shape', o.shape, o.dtype)


# revision 18
# speedup vs baseline: 1.0209x; 1.0209x over previous
"""DSV4 Main-KV projection kernel for 8 Trainium2 NeuronCores.

Computation (see reference): kv = x @ wkv.T ; RMSNorm(D=512) * rms_weight;
RoPE on last 64 dims; per-64-block fp8 quant-dequant simulation on first
448 dims. Data-parallel over the 16384 tokens (2048 per core).

x and wkv are cast to bf16 on the host (matmul rate on TRN2 is identical
to f32r at free-dim 512, but HBM traffic halves); all post-GEMM math stays
fp32. Inputs are pre-packed so every DMA is contiguous per partition.

Self-contained: hardcodes shapes; only imports the system toolchain.
"""
import sys
sys.path.insert(0, '/opt/trn_rl_repo')

import numpy as np
import ml_dtypes
from contextlib import ExitStack

import concourse.bass as bass
import concourse.mybir as mybir
import concourse.tile as tile
from concourse.bass_utils import run_bass_kernel_spmd
import bass_rust

dt = mybir.dt
BF16 = ml_dtypes.bfloat16

B, S, H, D = 4, 4096, 4096, 512
RD = 64                 # rope dims (last)
QD = D - RD             # quantized dims (first 448)
NBLK = QD // 64         # 7 quant blocks
BLK = 64
FP8_MAX = 448.0
EPS = 1e-6
ROPE_BASE = 10000.0
NCORES = 8
TOK = (B * S) // NCORES          # 2048 tokens per core
TT = 128                         # tokens per tile
NT = TOK // TT                   # 16 tiles per core
KC = H // 128                    # 32 contraction chunks
M_RND = 12582912.0               # 1.5 * 2**23: fp32 round-to-int magic

_compiled = {}


# ---------------------------------------------------------------------------
# walrus in this container caps sync waits at 1/instruction (2 for
# EventSemaphore); Tile emits more. Split the excess into preceding
# single-wait NoOps on the same engine.
def _split_multi_waits(nc):
    ctr = 0
    for f in nc.m.functions:
        for b in f.blocks:
            out, changed = [], False
            for inst in b.instructions:
                si = inst.sync_info
                cap = 2 if type(inst).__name__ == 'InstEventSemaphore' else 1
                if si is not None and len(si.on_wait) > cap:
                    waits = list(si.on_wait)
                    for w in waits[:-cap]:
                        ctr += 1
                        nop = mybir.InstNoOp(name=f'wsplit-{ctr}', ins=[], outs=[])
                        nop.engine = inst.engine
                        nop.sync_info = bass_rust.SyncInfo(on_wait=[w], on_update=[])
                        out.append(nop)
                    inst.sync_info = bass_rust.SyncInfo(on_wait=waits[-cap:],
                                                        on_update=si.on_update)
                    changed = True
                out.append(inst)
            if changed:
                b.instructions = out
    return ctr


def _build_nc(reps=1):
    nc = bass.Bass('TRN2', target_bir_lowering=False, debug=False)
    Alu = mybir.AluOpType
    Act = mybir.ActivationFunctionType

    # pre-packed inputs: contiguous per partition (see _host_prep)
    xb = nc.dram_tensor('xb', [128, NT * KC * TT], dt.bfloat16,
                        kind='ExternalInput').ap()
    wb = nc.dram_tensor('wb', [128, KC * D], dt.bfloat16,
                        kind='ExternalInput').ap()
    rmsr = nc.dram_tensor('rmsr', [128, D], dt.float32, kind='ExternalInput').ap()
    c2d = nc.dram_tensor('c2d', [128, NT * RD], dt.float32,
                         kind='ExternalInput').ap()
    s2d = nc.dram_tensor('s2d', [128, NT * RD], dt.float32,
                         kind='ExternalInput').ap()
    out = nc.dram_tensor('out', [TOK, D], dt.bfloat16, kind='ExternalOutput').ap()

    with tile.TileContext(nc) as tc, ExitStack() as ctx:
        const = ctx.enter_context(tc.tile_pool(name='const', bufs=1))
        xpool = ctx.enter_context(tc.tile_pool(name='xp', bufs=3))
        kpool = ctx.enter_context(tc.tile_pool(name='kp', bufs=3))
        opool = ctx.enter_context(tc.tile_pool(name='op', bufs=3))
        spool = ctx.enter_context(tc.tile_pool(name='sp', bufs=2))
        psum = ctx.enter_context(tc.tile_pool(name='ps', bufs=8, space='PSUM'))

        def xt_dma(t, name=None):
            # x tile [128, KC, TT] bf16 — one contiguous 8KB chunk/partition
            xt = xpool.tile([128, KC, TT], dt.bfloat16,
                            name=name or f'xt_{t}', tag='xt')
            nc.sync.dma_start(
                xt[:], xb[:, t * KC * TT:(t + 1) * KC * TT]
                .rearrange('p (c m) -> p c m', c=KC))
            return xt

        # DMA issue is split across the two HW-DGE engines: descriptor
        # generation costs ~0.6us per dma_start, serial per engine. Weights +
        # tables go through the Activation engine (idle at start); x tiles and
        # outputs through Sync. Weight chunks of 4 k-slices pace tile 0's
        # matmuls while the weight stream is still arriving.
        WCH = 2                           # k-slices per weight DMA chunk
        xt0 = xt_dma(0)
        wch = []
        for c in range(KC // WCH):
            wc = const.tile([128, WCH, D], dt.bfloat16, name=f'wc{c}')
            nc.scalar.dma_start(
                wc[:], wb[:, c * WCH * D:(c + 1) * WCH * D]
                .rearrange('p (j d) -> p j d', j=WCH))
            wch.append(wc)
        wts = [wch[k // WCH][:, k % WCH, :] for k in range(KC)]
        # replicated rms weight [128, D]
        rms = const.tile([128, D], dt.float32, name='rms')
        nc.scalar.dma_start(rms[:], rmsr)
        # rope tables [128, NT, RD]
        c2 = const.tile([128, NT, RD], dt.float32, name='c2')
        nc.scalar.dma_start(c2[:], c2d.rearrange('p (t f) -> p t f', t=NT))
        s2 = const.tile([128, NT, RD], dt.float32, name='s2')
        nc.scalar.dma_start(s2[:], s2d.rearrange('p (t f) -> p t f', t=NT))
        xt1 = xt_dma(1)

        for rep in range(reps):
         for t in range(NT):
             if rep == 0 and t == 0:
                 xap = lambda k, _x=xt0: _x[:, k, :]
             elif rep == 0 and t == 1:
                 xap = lambda k, _x=xt1: _x[:, k, :]
             else:
                 xt = xt_dma(t, name=f'xt_{rep}_{t}')
                 xap = lambda k, _x=xt: _x[:, k, :]

             ps = psum.tile([TT, D], dt.float32, name=f'ps{rep}_{t}', tag='ps')
             for k in range(KC):
                 nc.tensor.matmul(ps[:], xap(k), wts[k],
                                  start=(k == 0), stop=(k == KC - 1))

             # --- RMSNorm ---  var = mean(kv^2) via scale-folded Square;
             # EPS (1e-6) is negligible against var ~ 1 and is dropped.
             sq = spool.tile([TT, D], dt.float32, name=f'sq{rep}_{t}', tag='sq')
             var = spool.tile([TT, 1], dt.float32, name=f'var{rep}_{t}', tag='var')
             nc.scalar.activation(sq[:], ps[:], Act.Square,
                                  scale=float(1.0 / np.sqrt(D)), accum_out=var[:])
             rv = spool.tile([TT, 1], dt.float32, name=f'rv{rep}_{t}', tag='rv')
             nc.vector.reciprocal(rv[:], var[:])
             rstd = spool.tile([TT, 1], dt.float32, name=f'rstd{rep}_{t}', tag='rstd')
             nc.scalar.activation(rstd[:], rv[:], Act.Sqrt)
             # kvw = (kv * rstd) * rms
             kvw = kpool.tile([TT, D], dt.float32, name=f'kvw{rep}_{t}', tag='kvw')
             nc.vector.scalar_tensor_tensor(kvw[:], ps[:], rstd[:], rms[:],
                                            op0=Alu.mult, op1=Alu.mult)

             ot = opool.tile([TT, D], dt.bfloat16, name=f'ot{rep}_{t}', tag='ot')

             # --- quant-dequant on [:, :448] ---
             amax = spool.tile([TT, NBLK], dt.float32, name=f'amax{rep}_{t}', tag='amax')
             nc.vector.tensor_reduce(
                 amax[:], kvw[:, 0:QD].rearrange('p (b k) -> p b k', k=BLK),
                 axis=mybir.AxisListType.X, op=Alu.max, apply_absolute_value=True)
             sc = spool.tile([TT, NBLK], dt.float32, name=f'sc{rep}_{t}', tag='sc')
             nc.vector.tensor_scalar(sc[:], amax[:], 1e-4, FP8_MAX / 127.0 / FP8_MAX,
                                     op0=Alu.max, op1=Alu.mult)  # s = amax'/127
             rq = spool.tile([TT, NBLK], dt.float32, name=f'rq{rep}_{t}', tag='rq')
             nc.vector.reciprocal(rq[:], sc[:])                  # 127/amax'
             vq = kpool.tile([TT, QD], dt.float32, name=f'vq{rep}_{t}', tag='vq')
             rq_b = bass.AP(tensor=rq.tensor, offset=rq[:].offset,
                            ap=[[rq[:].ap[0][0], TT], [1, NBLK], [0, BLK]])
             nc.vector.tensor_tensor(
                 vq[:].rearrange('p (b k) -> p b k', k=BLK),
                 kvw[:, 0:QD].rearrange('p (b k) -> p b k', k=BLK),
                 rq_b, op=Alu.mult)
             nc.vector.tensor_scalar(vq[:], vq[:], M_RND, M_RND,
                                     op0=Alu.add, op1=Alu.subtract)
             sc_b = bass.AP(tensor=sc.tensor, offset=sc[:].offset,
                            ap=[[sc[:].ap[0][0], TT], [1, NBLK], [0, BLK]])
             nc.vector.tensor_tensor(
                 ot[:, 0:QD].rearrange('p (b k) -> p b k', k=BLK),
                 vq[:].rearrange('p (b k) -> p b k', k=BLK),
                 sc_b, op=Alu.mult)

             # --- rope on [:, 448:] (on GpSimd, concurrent with quant on DVE) ---
             # out = kvw_rope * c2 + pairswap(kvw_rope) * s2
             sw = spool.tile([TT, RD], dt.float32, name=f'sw{rep}_{t}', tag='sw')
             src_swap = bass.AP(tensor=kvw.tensor, offset=kvw[:].offset + QD + 1,
                                ap=[[kvw[:].ap[0][0], TT], [2, RD // 2], [-1, 2]])
             nc.gpsimd.tensor_copy(sw[:].rearrange('p (a b) -> p a b', b=2), src_swap)
             t1 = spool.tile([TT, RD], dt.float32, name=f't1{rep}_{t}', tag='t1')
             nc.gpsimd.tensor_tensor(t1[:], kvw[:, QD:D], c2[:, t, :], op=Alu.mult)
             t2 = spool.tile([TT, RD], dt.float32, name=f't2{rep}_{t}', tag='t2')
             nc.gpsimd.tensor_tensor(t2[:], sw[:], s2[:, t, :], op=Alu.mult)
             nc.gpsimd.tensor_tensor(ot[:, QD:D], t1[:], t2[:], op=Alu.add)

             nc.sync.dma_start(out[t * TT:(t + 1) * TT, :], ot[:])

    _split_multi_waits(nc)
    return nc


def _host_prep(x, wkv_weight, rms_weight):
    """Shard + pack on host; build rope tables. Returns per-core in_maps.

    Packed layouts (all contiguous per partition):
      xb [128, NT*KC*TT] bf16 : xb[p, (t*KC+c)*TT+m] = x[tok0 + t*TT+m, c*128+p]
      wb [128, KC*D]     bf16 : wb[p, c*D+d]         = wkv[d, c*128+p]
      c2d/s2d [128, NT*RD] f32: c2d[p, t*RD+f]       = table[pos(t*TT+p), f]
    """
    xf = np.ascontiguousarray(x, dtype=np.float32).reshape(B * S, H)
    wb = np.ascontiguousarray(
        wkv_weight.astype(np.float32).T.reshape(KC, 128, D).transpose(1, 0, 2)
        .astype(BF16).reshape(128, KC * D))
    rmsr = np.broadcast_to(np.asarray(rms_weight, np.float32)[None, :],
                           (128, D)).copy()

    # rope tables for all positions: duplicated cos / sign-folded sin
    freqs = 1.0 / ROPE_BASE ** (np.arange(0, RD, 2, dtype=np.float64) / RD)
    tpos = np.arange(S, dtype=np.float64)
    ang = np.outer(tpos, freqs)                                        # [S, 32]
    cos = np.cos(ang).astype(np.float32)
    sin = np.sin(ang).astype(np.float32)
    c2 = np.empty((S, RD), np.float32)
    s2 = np.empty((S, RD), np.float32)
    c2[:, 0::2] = cos
    c2[:, 1::2] = cos
    s2[:, 0::2] = -sin          # even out: a*cos - b*sin ; sw[even]=b
    s2[:, 1::2] = sin           # odd  out: a*sin + b*cos ; sw[odd]=a

    in_maps = []
    for c in range(NCORES):
        tok0 = c * TOK
        # [TOK, H] -> [t, m, kc, p] -> [p, t, kc, m] bf16, contiguous
        xs = (xf[tok0:tok0 + TOK, :].reshape(NT, TT, KC, 128)
              .transpose(3, 0, 2, 1).astype(BF16).reshape(128, NT * KC * TT))
        spos = (np.arange(tok0, tok0 + TOK)) % S
        c2c = np.ascontiguousarray(
            c2[spos].reshape(NT, TT, RD).transpose(1, 0, 2)
            .reshape(128, NT * RD))
        s2c = np.ascontiguousarray(
            s2[spos].reshape(NT, TT, RD).transpose(1, 0, 2)
            .reshape(128, NT * RD))
        in_maps.append({
            'xb': xs,
            'wb': wb,
            'rmsr': rmsr,
            'c2d': c2c,
            's2d': s2c,
        })
    return in_maps


def kernel(x, wkv_weight, rms_weight, _trace=False, _trace_kwargs=None):
    in_maps = _host_prep(x, wkv_weight, rms_weight)
    if 'nc' not in _compiled:
        _compiled['nc'] = _build_nc()
    nc = _compiled['nc']
    kw = {}
    if _trace:
        kw = dict(trace=True, trace_cores=[0], **(_trace_kwargs or {}))
    res = run_bass_kernel_spmd(nc, in_maps, core_ids=list(range(NCORES)), **kw)
    outs = [r['out'] for r in res.results]
    full = np.concatenate(outs, axis=0).reshape(B, S, D).astype(np.float32)
    kernel._last_results = res
    return full


if __name__ == '__main__':
    rng = np.random.default_rng(0)
    x = rng.standard_normal((B, S, H), dtype=np.float32)
    w = (rng.standard_normal((D, H), dtype=np.float32) * H ** -0.5).astype(np.float32)
    rw = np.ones((D,), np.float32)
    o = kernel(x, w, rw)
    print('out shape', o.shape, o.dtype)
